# revision 22
# baseline (speedup 1.0000x reference)
"""Trainium2 Bass kernel: 3-layer edge-weighted GraphSAGE (Cluster-GCN style).

Strategy (8 NeuronCores, SPMD):
  - Nodes padded to NPAD = 8*SHARD, shard k = rows [k*SHARD, (k+1)*SHARD).
  - Edges sorted by (dst tile, src half); per dst-tile (128 nodes) the
    incoming edges' src rows are gathered with dma_gather (int16 indices,
    one call per (tile, half), rotated over the 4 SWDGE queues), then
    aggregated with a one-hot matmul into PSUM.
  - The one-hot selection matrices S[e, n] = (dst_e == n) * w'_e (with
    w' = edge_attr / max(indeg, 1), folding the mean) are identical for all
    three layers and are PRECOMPUTED ON THE HOST, uploaded to DRAM, and
    streamed into SBUF per chunk group (split into two DMAs per group so the
    large streaming descriptors don't head-of-line-block gather descriptors
    on the DMA engines).
  - A src node's "half" is whether it falls in the first or second half of
    its OWNING core's shard.  Each layer boundary then uses TWO AllGathers:
    AG_A for the first halves (issued mid-layer, as soon as the first
    TPC/2 tiles are done) and AG_B at the end of the layer.  The next
    layer runs in two passes: pass A aggregates only first-half chunks
    (tables ready after AG_A, so it overlaps AG_B), spilling the partial
    PSUM to SBUF; pass B reloads the partial via an identity matmul and
    finishes.  This hides nearly all collective time under gather work.
  - Layer 3 projects h2 @ Wl2 first (8 cols, padded to 256B rows) so its
    gather moves 256B/edge instead of 1KB/edge.
  - PSUM->SBUF copies are split between the Scalar engine (Act Copy only,
    so the activation table is loaded once) and the DVE; ReLU runs on the
    DVE (max with 0).  Layer 3's log_softmax skips the max-subtraction
    (|logits| is small) and batches: per-tile Exp with accumulate, batched
    Ln (avoids per-tile activation-table reloads at 1.28us each).
  - Full chunks are always gathered (padding slots -> row 0) so no SBUF
    garbage (possible NaN) reaches the PSUM accumulation through the zero
    columns of S.
  - bf16 matmul operands (fp32 PSUM accumulation) for full PE rate and
    half gather bandwidth.
"""
import numpy as np

import concourse.bacc as bacc
import concourse.tile as tile
from concourse import mybir
from concourse.bass_utils import run_bass_kernel_spmd
from concourse.masks import make_identity

from ml_dtypes import bfloat16 as np_bf16
from ml_dtypes import float8_e4m3fn as np_f8

F32 = mybir.dt.float32
BF16 = mybir.dt.bfloat16
F8 = mybir.dt.float8e4
I16 = mybir.dt.int16
P = 128
Alu = mybir.AluOpType
Act = mybir.ActivationFunctionType


class Cfg:
    def __init__(self, n_nodes=50000, n_edges=800000, dims=(128, 256, 256, 8),
                 ncores=8, G=2, bf16=True):
        self.N, self.E, self.D, self.NC = n_nodes, n_edges, dims, ncores
        self.SHARD = ((n_nodes + ncores * P - 1) // (ncores * P)) * P
        self.NPAD = self.SHARD * ncores
        self.TPC = self.SHARD // P
        self.TMID = (self.TPC + 1) // 2  # tiles in the A half
        self.SH2A = self.TMID * P
        self.SH2B = self.SHARD - self.SH2A
        assert self.NC * self.SH2A < 32768
        assert dims[0] == P and dims[1] % P == 0 and dims[2] % P == 0
        self.G, self.bf16 = G, bf16
        # L3 gather table row width (256B rows)
        self.EL3 = 128 if bf16 else 64

    def key(self):
        return (self.N, self.E, self.D, self.NC, self.G, self.bf16)


class PlanH:
    """Chunk layout for one src-half: per-tile chunk bases, group bases."""

    def __init__(self, cfg, nab):
        self.nab = nab  # [TPC] chunks per tile
        self.groups = [list(range(i, min(cfg.TPC, i + cfg.G)))
                       for i in range(0, cfg.TPC, cfg.G)]
        self.cb = np.zeros(cfg.TPC, np.int64)
        self.gbase, self.gc = [], []
        c = 0
        for tl in self.groups:
            self.gbase.append(c)
            for t in tl:
                self.cb[t] = c
                c += nab[t]
            self.gc.append(c - self.gbase[-1])
        self.CT = c


def host_prep(cfg, edge_index, edge_attr):
    src = edge_index[0].astype(np.int64)
    dst = edge_index[1].astype(np.int64)
    cnt = np.bincount(dst, minlength=cfg.N).astype(np.float32)
    wp = (edge_attr.astype(np.float32)
          / np.maximum(cnt, 1.0)[dst]).astype(np.float32)

    loc = src % cfg.SHARD
    hsel = (loc >= cfg.SH2A).astype(np.int64)
    row = np.where(hsel == 0,
                   (src // cfg.SHARD) * cfg.SH2A + loc,
                   (src // cfg.SHARD) * cfg.SH2B + loc - cfg.SH2A)
    segkey = (dst >> 7) * 2 + hsel
    order = np.argsort(segkey, kind="stable")
    srow, sdst, swp = row[order], dst[order], wp[order]
    nseg = (cfg.NPAD // P) * 2
    seg_counts = np.bincount(segkey, minlength=nseg)
    seg_start = np.zeros(nseg + 1, np.int64)
    seg_start[1:] = np.cumsum(seg_counts)
    sc = seg_counts.reshape(cfg.NC, cfg.TPC, 2)
    nabAB = np.maximum(np.ceil(sc / P).astype(np.int64).max(axis=0), 1)
    plans = (PlanH(cfg, nabAB[:, 0]), PlanH(cfg, nabAB[:, 1]))

    idxs, ss = [], []
    for h in (0, 1):
        plan = plans[h]
        CT = plan.CT
        idx_arr = np.zeros((cfg.NC, 16, CT * 8), np.int16)
        s_arr = np.zeros((cfg.NC, P, CT, P), np.float32)
        for k in range(cfg.NC):
            for t in range(cfg.TPC):
                si = (k * cfg.TPC + t) * 2 + h
                i0, n = seg_start[si], seg_counts[si]
                if n == 0:
                    continue
                rows = srow[i0:i0 + n].astype(np.int16)
                stbase = k * cfg.SHARD + t * P
                dl = (sdst[i0:i0 + n] - stbase).astype(np.int64)
                cb = plan.cb[t]
                j = np.arange(n)
                s_arr[k, j % P, cb + j // P, dl] = swp[i0:i0 + n]
                idx_arr[k, j % 16, cb * 8 + j // 16] = rows
        idxs.append(np.tile(idx_arr, (1, 8, 1)))
        ss.append(s_arr)
    return plans, idxs, ss


def build_nc(cfg, plans):
    d0, d1, d2, d3 = cfg.D
    H1, H2 = d1 // P, d2 // P
    DT = BF16 if cfg.bf16 else F32
    planA, planB = plans
    NHA = cfg.NC * cfg.SH2A
    NHB = cfg.NC * cfg.SH2B

    nc = bacc.Bacc("TRN2", target_bir_lowering=False, debug=False,
                   num_devices=cfg.NC, enable_asserts=False,
                   num_swdge_queues=4)

    xA_in = nc.dram_tensor("xA", [NHA, d0], DT, kind="ExternalInput")
    xB_in = nc.dram_tensor("xB", [NHB, d0], DT, kind="ExternalInput")
    xT_in = nc.dram_tensor("xT", [P, cfg.SHARD], DT, kind="ExternalInput")
    idxA_in = nc.dram_tensor("idxA", [P, planA.CT * 8], I16,
                             kind="ExternalInput")
    idxB_in = nc.dram_tensor("idxB", [P, planB.CT * 8], I16,
                             kind="ExternalInput")
    sA_in = nc.dram_tensor("sA", [P, planA.CT, P], F8, kind="ExternalInput")
    sB_in = nc.dram_tensor("sB", [P, planB.CT, P], F8, kind="ExternalInput")
    wl0_in = nc.dram_tensor("wl0", [P, d1], DT, kind="ExternalInput")
    wr0_in = nc.dram_tensor("wr0", [P, d1], DT, kind="ExternalInput")
    wl1_in = nc.dram_tensor("wl1", [P, H1, d2], DT, kind="ExternalInput")
    wr1_in = nc.dram_tensor("wr1", [P, H1, d2], DT, kind="ExternalInput")
    wl2_in = nc.dram_tensor("wl2", [P, H2, d3], DT, kind="ExternalInput")
    wr2_in = nc.dram_tensor("wr2", [P, H2, d3], DT, kind="ExternalInput")
    b0_in = nc.dram_tensor("b0", [1, d1], DT, kind="ExternalInput")
    b1_in = nc.dram_tensor("b1", [1, d2], DT, kind="ExternalInput")
    b2_in = nc.dram_tensor("b2", [1, d3], DT, kind="ExternalInput")
    out_t = nc.dram_tensor("out", [cfg.SHARD, d3], F32, kind="ExternalOutput")

    with tile.TileContext(nc) as tc:
        with (
            tc.tile_pool(name="const", bufs=1) as cp,
            tc.tile_pool(name="mt", bufs=3) as mp,
            tc.tile_pool(name="st", bufs=3) as sp,
            tc.tile_pool(name="wk", bufs=3) as wk,
            tc.tile_pool(name="psum", bufs=2, space="PSUM") as ps,
            tc.tile_pool(name="dram", bufs=1, space="DRAM") as dr,
        ):
            h1ownA = dr.tile([cfg.SH2A, d1], F8)
            h1ownB = dr.tile([cfg.SH2B, d1], F8)
            h1fullA = dr.tile([NHA, d1], F8, addr_space="Shared")
            h1fullB = dr.tile([NHB, d1], F8, addr_space="Shared")
            h1T = dr.tile([d1, cfg.SHARD], DT)
            h2T = dr.tile([d2, cfg.SHARD], DT)
            h2pA = dr.tile([cfg.SH2A, cfg.EL3], DT)
            h2pB = dr.tile([cfg.SH2B, cfg.EL3], DT)
            h2pfullA = dr.tile([NHA, cfg.EL3], DT, addr_space="Shared")
            h2pfullB = dr.tile([NHB, cfg.EL3], DT, addr_space="Shared")

            # ---- constants / parameters
            ident_f = cp.tile([P, P], F32)
            make_identity(nc, ident_f[:])
            if cfg.bf16:
                ident_b = cp.tile([P, P], BF16)
                nc.vector.tensor_copy(ident_b[:], ident_f[:])
                ident_dt = ident_b
            else:
                ident_dt = ident_f
            ones_t = cp.tile([1, P], DT)
            nc.vector.memset(ones_t[:], 1.0)
            xT_t = cp.tile([P, cfg.SHARD], DT)
            nc.sync.dma_start(out=xT_t[:], in_=xT_in[:, :])
            idxA_t = cp.tile([P, planA.CT * 8], I16)
            nc.sync.dma_start(out=idxA_t[:], in_=idxA_in[:, :])
            idxB_t = cp.tile([P, planB.CT * 8], I16)
            nc.sync.dma_start(out=idxB_t[:], in_=idxB_in[:, :])
            wl0_t = cp.tile([P, d1], DT)
            nc.sync.dma_start(out=wl0_t[:], in_=wl0_in[:, :])
            wr0_t = cp.tile([P, d1], DT)
            nc.sync.dma_start(out=wr0_t[:], in_=wr0_in[:, :])
            wl1_t = cp.tile([P, H1, d2], DT)
            nc.sync.dma_start(out=wl1_t[:], in_=wl1_in[:, :, :])
            wr1_t = cp.tile([P, H1, d2], DT)
            nc.sync.dma_start(out=wr1_t[:], in_=wr1_in[:, :, :])
            wl2_t = cp.tile([P, H2, d3], DT)
            nc.sync.dma_start(out=wl2_t[:], in_=wl2_in[:, :, :])
            wr2_t = cp.tile([P, H2, d3], DT)
            nc.sync.dma_start(out=wr2_t[:], in_=wr2_in[:, :, :])
            b0_t = cp.tile([1, d1], DT)
            nc.sync.dma_start(out=b0_t[:], in_=b0_in[:, :])
            b1_t = cp.tile([1, d2], DT)
            nc.sync.dma_start(out=b1_t[:], in_=b1_in[:, :])
            b2_t = cp.tile([1, d3], DT)
            nc.sync.dma_start(out=b2_t[:], in_=b2_in[:, :])

            qctr = [0]  # round-robin SWDGE queue counter

            def gather_half(plan, idx_t, s_in, gi, table, elem, suf,
                            mdt=DT):
                gc = plan.gc[gi]
                gb = plan.gbase[gi]
                m_t = mp.tile([P, gc, elem], mdt, tag="mt" + suf)
                s_t = sp.tile([P, gc, P], F8, tag="st" + suf)
                hc = max(gc // 2, 1)
                nc.sync.dma_start(out=s_t[:, 0:hc, :],
                                  in_=s_in[:, gb:gb + hc, :])
                if gc > hc:
                    nc.sync.dma_start(out=s_t[:, hc:gc, :],
                                      in_=s_in[:, gb + hc:gb + gc, :])
                nidx = gc * P
                nc.gpsimd.dma_gather(
                    m_t[:, :, :], table,
                    idx_t[:, gb * 8:gb * 8 + gc * 8],
                    nidx, nidx, elem, single_packet=False,
                    queue_num=qctr[0] % 4)
                qctr[0] += 1
                return m_t, s_t

            def ag(src_ap, dst_tile):
                nc.gpsimd.collective_compute(
                    "AllGather", Alu.bypass,
                    replica_groups=[list(range(cfg.NC))],
                    ins=[src_ap], outs=[dst_tile.opt()])

            TMID = cfg.TMID  # tiles 0..TMID-1 land in the A half

            # ---------------- Layer 1 ----------------
            ag1a_done = False
            for gi, tl in enumerate(planA.groups):
                mA, sAt = gather_half(planA, idxA_t, sA_in, gi,
                                      xA_in[:, :], d0, "A")
                mB, sBt = gather_half(planB, idxB_t, sB_in, gi,
                                      xB_in[:, :], d0, "B")
                gbA, gbB = planA.gbase[gi], planB.gbase[gi]
                for t in tl:
                    aggT = ps.tile([P, P], F32, tag="agg")
                    mm = ([(mA, sAt, c - gbA) for c in
                           range(planA.cb[t], planA.cb[t] + planA.nab[t])]
                          + [(mB, sBt, c - gbB) for c in
                             range(planB.cb[t], planB.cb[t] + planB.nab[t])])
                    for ci, (m_t, s_t, c) in enumerate(mm):
                        nc.tensor.matmul(
                            out=aggT[:], lhsT=m_t[:, c, :], rhs=s_t[:, c, :],
                            start=(ci == 0), stop=(ci == len(mm) - 1))
                    meanT = wk.tile([P, P], DT, tag="meanT")
                    nc.scalar.activation(meanT[:], aggT[:], Act.Copy)
                    op_ = ps.tile([P, d1], F32, tag="outp")
                    nc.tensor.matmul(out=op_[:], lhsT=meanT[:],
                                     rhs=wl0_t[:], start=True, stop=False)
                    nc.tensor.matmul(out=op_[:],
                                     lhsT=xT_t[:, t * P:(t + 1) * P],
                                     rhs=wr0_t[:], start=False, stop=False)
                    nc.tensor.matmul(out=op_[:], lhsT=ones_t[:],
                                     rhs=b0_t[:], start=False,
                                     stop=True, skip_group_check=True)
                    h_sb = wk.tile([P, d1], DT, tag="h_sb")
                    nc.vector.tensor_scalar(out=h_sb[:], in0=op_[:],
                                            scalar1=0.0, scalar2=None,
                                            op0=Alu.max)  # relu + cast
                    h8_sb = wk.tile([P, d1], F8, tag="h8")
                    nc.vector.tensor_scalar(out=h8_sb[:], in0=op_[:],
                                            scalar1=0.0, scalar2=None,
                                            op0=Alu.max)  # relu + fp8 cast
                    hodst, hor = ((h1ownA, t) if t < TMID
                                  else (h1ownB, t - TMID))
                    nc.sync.dma_start(
                        out=hodst[hor * P:(hor + 1) * P, :], in_=h8_sb[:])
                    for hh in range(H1):
                        trp = ps.tile([P, P], DT, tag="trp")
                        nc.tensor.transpose(
                            out=trp[:], in_=h_sb[:, hh * P:(hh + 1) * P],
                            identity=ident_dt[:])
                        hT_sb = wk.tile([P, P], DT, tag="hT_sb")
                        nc.vector.tensor_copy(hT_sb[:], trp[:])
                        nc.sync.dma_start(
                            out=h1T[hh * P:(hh + 1) * P, t * P:(t + 1) * P],
                            in_=hT_sb[:])
                if not ag1a_done and tl[-1] >= TMID - 1:
                    ag(h1ownA.opt(), h1fullA)
                    ag1a_done = True
            ag(h1ownB.opt(), h1fullB)

            # ---------------- Layer 2 ----------------
            accA2 = cp.tile([P, cfg.TPC, d1], DT)
            # pass A: first-half chunks only (overlaps AG1_B)
            for gi, tl in enumerate(planA.groups):
                mA, sAt = gather_half(planA, idxA_t, sA_in, gi,
                                      h1fullA[:, :], d1, "A", mdt=F8)
                gbA = planA.gbase[gi]
                for t in tl:
                    agg = ps.tile([P, d1], F32, tag="outp")
                    nab = int(planA.nab[t])
                    for ci in range(nab):
                        c = planA.cb[t] - gbA + ci
                        nc.tensor.matmul(
                            out=agg[:], lhsT=sAt[:, c, :], rhs=mA[:, c, :],
                            start=(ci == 0), stop=(ci == nab - 1))
                    nc.scalar.activation(accA2[:, t, :], agg[:], Act.Copy)
            # pass B: reload partial, add second-half chunks, finish layer
            ag2a_done = False
            for gi, tl in enumerate(planB.groups):
                mB, sBt = gather_half(planB, idxB_t, sB_in, gi,
                                      h1fullB[:, :], d1, "B", mdt=F8)
                gbB = planB.gbase[gi]
                for t in tl:
                    agg = ps.tile([P, d1], F32, tag="outp")
                    nc.tensor.matmul(out=agg[:], lhsT=ident_dt[:],
                                     rhs=accA2[:, t, :], start=True,
                                     stop=False)
                    nab = int(planB.nab[t])
                    for ci in range(nab):
                        c = planB.cb[t] - gbB + ci
                        nc.tensor.matmul(
                            out=agg[:], lhsT=sBt[:, c, :], rhs=mB[:, c, :],
                            start=False, stop=(ci == nab - 1))
                    agg_sb = wk.tile([P, d1], DT, tag="agg_sb")
                    nc.scalar.activation(agg_sb[:], agg[:], Act.Copy)
                    mts = []
                    for hh in range(H1):
                        trp = ps.tile([P, P], DT, tag="trp")
                        nc.tensor.transpose(
                            out=trp[:], in_=agg_sb[:, hh * P:(hh + 1) * P],
                            identity=ident_dt[:])
                        mt_sb = wk.tile([P, P], DT, tag="mT2")
                        nc.scalar.activation(mt_sb[:], trp[:], Act.Copy)
                        mts.append(mt_sb)
                    h1T_t = wk.tile([P, H1, P], DT, tag="hTt")
                    for hh in range(H1):
                        nc.sync.dma_start(
                            out=h1T_t[:, hh, :],
                            in_=h1T[hh * P:(hh + 1) * P, t * P:(t + 1) * P])
                    op_ = ps.tile([P, d2], F32, tag="outp")
                    for hh in range(H1):
                        nc.tensor.matmul(out=op_[:], lhsT=mts[hh][:],
                                         rhs=wl1_t[:, hh, :],
                                         start=(hh == 0), stop=False)
                    for hh in range(H1):
                        nc.tensor.matmul(out=op_[:], lhsT=h1T_t[:, hh, :],
                                         rhs=wr1_t[:, hh, :],
                                         start=False, stop=False)
                    nc.tensor.matmul(out=op_[:], lhsT=ones_t[:],
                                     rhs=b1_t[:], start=False, stop=True,
                                     skip_group_check=True)
                    h_sb = wk.tile([P, d2], DT, tag="h_sb")
                    nc.vector.tensor_scalar(out=h_sb[:], in0=op_[:],
                                            scalar1=0.0, scalar2=None,
                                            op0=Alu.max)  # relu + cast
                    hts = []
                    for hh in range(H2):
                        trp = ps.tile([P, P], DT, tag="trp")
                        nc.tensor.transpose(
                            out=trp[:], in_=h_sb[:, hh * P:(hh + 1) * P],
                            identity=ident_dt[:])
                        hT_sb = wk.tile([P, P], DT, tag="hT_sb")
                        nc.vector.tensor_copy(hT_sb[:], trp[:])
                        nc.sync.dma_start(
                            out=h2T[hh * P:(hh + 1) * P, t * P:(t + 1) * P],
                            in_=hT_sb[:])
                        hts.append(hT_sb)
                    prj = ps.tile([P, d3], F32, tag="proj")
                    for hh in range(H2):
                        nc.tensor.matmul(out=prj[:], lhsT=hts[hh][:],
                                         rhs=wl2_t[:, hh, :],
                                         start=(hh == 0), stop=(hh == H2 - 1))
                    prj_sb = wk.tile([P, d3], DT, tag="prj_sb")
                    nc.vector.tensor_copy(prj_sb[:], prj[:])
                    hpdst, hpr = ((h2pA, t) if t < TMID
                                  else (h2pB, t - TMID))
                    nc.sync.dma_start(
                        out=hpdst[hpr * P:(hpr + 1) * P, 0:d3],
                        in_=prj_sb[:])
                if not ag2a_done and tl[-1] >= TMID - 1:
                    ag(h2pA.opt(), h2pfullA)
                    ag2a_done = True
            ag(h2pB.opt(), h2pfullB)

            # ---------------- Layer 3 ----------------
            # log_softmax without max-subtraction (logit scale is small):
            # z - ln(sum(exp(z))); Exp accumulates per tile, batched Ln
            se_all = cp.tile([P, cfg.TPC], F32)
            z_all = cp.tile([P, cfg.TPC, d3], F32)
            ls_all = cp.tile([P, cfg.TPC], F32)
            accA3 = cp.tile([P, cfg.TPC, d3], DT)

            def emit_tail(t0, t1):
                nc.scalar.activation(ls_all[:, t0:t1], se_all[:, t0:t1],
                                     Act.Ln)
                for t in range(t0, t1):
                    out_sb = wk.tile([P, d3], F32, tag="out_sb")
                    nc.vector.tensor_scalar(out=out_sb[:],
                                            in0=z_all[:, t, :],
                                            scalar1=ls_all[:, t:t + 1],
                                            scalar2=None, op0=Alu.subtract)
                    nc.sync.dma_start(out=out_t[t * P:(t + 1) * P, :],
                                      in_=out_sb[:])

            # pass A (overlaps AG2_B)
            for gi, tl in enumerate(planA.groups):
                mA, sAt = gather_half(planA, idxA_t, sA_in, gi,
                                      h2pfullA[:, :], cfg.EL3, "A")
                gbA = planA.gbase[gi]
                for t in tl:
                    op_ = ps.tile([P, d3], F32, tag="proj")
                    nab = int(planA.nab[t])
                    for ci in range(nab):
                        c = planA.cb[t] - gbA + ci
                        nc.tensor.matmul(
                            out=op_[:], lhsT=sAt[:, c, :], rhs=mA[:, c, 0:d3],
                            start=(ci == 0), stop=(ci == nab - 1))
                    nc.vector.tensor_copy(accA3[:, t, :], op_[:])
            # pass B
            TS1 = (3 * cfg.TPC // 5) // cfg.G * cfg.G
            TS2 = (9 * cfg.TPC // 10) // cfg.G * cfg.G
            for gi, tl in enumerate(planB.groups):
                mB, sBt = gather_half(planB, idxB_t, sB_in, gi,
                                      h2pfullB[:, :], cfg.EL3, "B")
                gbB = planB.gbase[gi]
                for t in tl:
                    op_ = ps.tile([P, d3], F32, tag="proj")
                    nc.tensor.matmul(out=op_[:], lhsT=ident_dt[:],
                                     rhs=accA3[:, t, :], start=True,
                                     stop=False)
                    nab = int(planB.nab[t])
                    for ci in range(nab):
                        c = planB.cb[t] - gbB + ci
                        nc.tensor.matmul(
                            out=op_[:], lhsT=sBt[:, c, :], rhs=mB[:, c, 0:d3],
                            start=False, stop=False)
                    h2T_t = wk.tile([P, H2, P], DT, tag="hTt")
                    for hh in range(H2):
                        nc.sync.dma_start(
                            out=h2T_t[:, hh, :],
                            in_=h2T[hh * P:(hh + 1) * P, t * P:(t + 1) * P])
                    for hh in range(H2):
                        nc.tensor.matmul(out=op_[:], lhsT=h2T_t[:, hh, :],
                                         rhs=wr2_t[:, hh, :],
                                         start=False, stop=False,
                                         skip_group_check=True)
                    nc.tensor.matmul(out=op_[:], lhsT=ones_t[:],
                                     rhs=b2_t[:], start=False, stop=True,
                                     skip_group_check=True)
                    nc.vector.tensor_copy(z_all[:, t, :], op_[:])
                    e_dummy = wk.tile([P, d3], F32, tag="e_sb")
                    nc.scalar.activation(e_dummy[:], op_[:], Act.Exp,
                                         accum_out=se_all[:, t:t + 1])
                if tl[-1] + 1 == TS1:
                    emit_tail(0, TS1)
                elif tl[-1] + 1 == TS2:
                    emit_tail(TS1, TS2)
            emit_tail(TS2, cfg.TPC)

    nc.compile()
    return nc


_NC_CACHE = {}


def get_nc(cfg, plans):
    key = (cfg.key(), plans[0].nab.tobytes(), plans[1].nab.tobytes())
    if key not in _NC_CACHE:
        _NC_CACHE[key] = build_nc(cfg, plans)
    return _NC_CACHE[key]


def run(cfg, inputs, trace=False, tmpdir=None):
    x = np.asarray(inputs["x"], np.float32)
    plans, idxs, ss = host_prep(
        cfg, np.asarray(inputs["edge_index"]),
        np.asarray(inputs["edge_attr"], np.float32))
    d0, d1, d2, d3 = cfg.D
    H1, H2 = d1 // P, d2 // P
    npDT = np_bf16 if cfg.bf16 else np.float32

    xpad = np.zeros((cfg.NPAD, d0), np.float32)
    xpad[:cfg.N] = x
    xpad = xpad.astype(npDT)
    xsh = xpad.reshape(cfg.NC, cfg.SHARD, d0)
    xA = np.ascontiguousarray(xsh[:, :cfg.SH2A].reshape(-1, d0))
    xB = np.ascontiguousarray(xsh[:, cfg.SH2A:].reshape(-1, d0))
    Wl1 = np.asarray(inputs["Wl1"], np.float32)
    Wr1 = np.asarray(inputs["Wr1"], np.float32)
    Wl2 = np.asarray(inputs["Wl2"], np.float32)
    Wr2 = np.asarray(inputs["Wr2"], np.float32)
    shared = {
        "xA": xA,
        "xB": xB,
        "wl0": np.asarray(inputs["Wl0"], np.float32).astype(npDT),
        "wr0": np.asarray(inputs["Wr0"], np.float32).astype(npDT),
        "wl1": Wl1.reshape(H1, P, d2).transpose(1, 0, 2).astype(npDT),
        "wr1": Wr1.reshape(H1, P, d2).transpose(1, 0, 2).astype(npDT),
        "wl2": Wl2.reshape(H2, P, d3).transpose(1, 0, 2).astype(npDT),
        "wr2": Wr2.reshape(H2, P, d3).transpose(1, 0, 2).astype(npDT),
        "b0": (np.asarray(inputs["bl0"]) + np.asarray(inputs["br0"]))
        .astype(np.float32)[None, :].astype(npDT),
        "b1": (np.asarray(inputs["bl1"]) + np.asarray(inputs["br1"]))
        .astype(np.float32)[None, :].astype(npDT),
        "b2": (np.asarray(inputs["bl2"]) + np.asarray(inputs["br2"]))
        .astype(np.float32)[None, :].astype(npDT),
    }
    in_maps = []
    for k in range(cfg.NC):
        in_maps.append({
            **shared,
            "xT": np.ascontiguousarray(
                xpad[k * cfg.SHARD:(k + 1) * cfg.SHARD].T),
            "idxA": idxs[0][k],
            "idxB": idxs[1][k],
            "sA": ss[0][k].astype(np_f8),
            "sB": ss[1][k].astype(np_f8),
        })
    nc = get_nc(cfg, plans)
    res = run_bass_kernel_spmd(nc, in_maps, core_ids=list(range(cfg.NC)),
                               trace=trace, tmpdir=tmpdir)
    out = np.concatenate([res.results[k]["out"] for k in range(cfg.NC)],
                         axis=0)[:cfg.N]
    return np.ascontiguousarray(out.astype(np.float32)), res


def kernel(**inputs):
    cfg = Cfg()
    out, _ = run(cfg, inputs)
    return out


# revision 23
# speedup vs baseline: 1.0837x; 1.0837x over previous
"""Trainium2 Bass kernel: 3-layer edge-weighted GraphSAGE (Cluster-GCN style).

Strategy (8 NeuronCores, SPMD):
  - Nodes padded to NPAD = 8*SHARD, shard k = rows [k*SHARD, (k+1)*SHARD).
  - Edges sorted by (dst tile, src half); per dst-tile (128 nodes) the
    incoming edges' src rows are gathered with dma_gather (int16 indices,
    one call per (tile, half), rotated over the 4 SWDGE queues), then
    aggregated with a one-hot matmul into PSUM.
  - The one-hot selection matrices S[e, n] = (dst_e == n) * w'_e (with
    w' = edge_attr / max(indeg, 1), folding the mean) are identical for all
    three layers and are PRECOMPUTED ON THE HOST, uploaded to DRAM, and
    streamed into SBUF per chunk group (split into two DMAs per group so the
    large streaming descriptors don't head-of-line-block gather descriptors
    on the DMA engines).
  - A src node's "half" is whether it falls in the first or second half of
    its OWNING core's shard.  Each layer boundary then uses TWO AllGathers:
    AG_A for the first halves (issued mid-layer, as soon as the first
    TPC/2 tiles are done) and AG_B at the end of the layer.  The next
    layer runs in two passes: pass A aggregates only first-half chunks
    (tables ready after AG_A, so it overlaps AG_B), spilling the partial
    PSUM to SBUF; pass B reloads the partial via an identity matmul and
    finishes.  This hides nearly all collective time under gather work.
  - Layer 3 projects h2 @ Wl2 first (8 cols, padded to 256B rows) so its
    gather moves 256B/edge instead of 1KB/edge.
  - PSUM->SBUF copies are split between the Scalar engine (Act Copy only,
    so the activation table is loaded once) and the DVE; ReLU runs on the
    DVE (max with 0).  Layer 3's log_softmax skips the max-subtraction
    (|logits| is small) and batches: per-tile Exp with accumulate, batched
    Ln (avoids per-tile activation-table reloads at 1.28us each).
  - Full chunks are always gathered (padding slots -> row 0) so no SBUF
    garbage (possible NaN) reaches the PSUM accumulation through the zero
    columns of S.
  - bf16 matmul operands (fp32 PSUM accumulation) for full PE rate and
    half gather bandwidth.
"""
import numpy as np

import concourse.bacc as bacc
import concourse.tile as tile
from concourse import mybir
from concourse.bass_utils import run_bass_kernel_spmd
from concourse.masks import make_identity

from ml_dtypes import bfloat16 as np_bf16
from ml_dtypes import float8_e4m3fn as np_f8

F32 = mybir.dt.float32
BF16 = mybir.dt.bfloat16
F8 = mybir.dt.float8e4
I16 = mybir.dt.int16
P = 128
Alu = mybir.AluOpType
Act = mybir.ActivationFunctionType


class Cfg:
    def __init__(self, n_nodes=50000, n_edges=800000, dims=(128, 256, 256, 8),
                 ncores=8, G=2, bf16=True):
        self.N, self.E, self.D, self.NC = n_nodes, n_edges, dims, ncores
        self.SHARD = ((n_nodes + ncores * P - 1) // (ncores * P)) * P
        self.NPAD = self.SHARD * ncores
        self.TPC = self.SHARD // P
        self.TMID = (self.TPC + 1) // 2  # tiles in the A half
        self.SH2A = self.TMID * P
        self.SH2B = self.SHARD - self.SH2A
        assert self.NC * self.SH2A < 32768
        assert dims[0] == P and dims[1] % P == 0 and dims[2] % P == 0
        self.G, self.bf16 = G, bf16
        # L3 gather table row width (256B rows)
        self.EL3 = 128 if bf16 else 64

    def key(self):
        return (self.N, self.E, self.D, self.NC, self.G, self.bf16)


class PlanH:
    """Chunk layout for one src-half: per-tile chunk bases, group bases."""

    def __init__(self, cfg, nab):
        self.nab = nab  # [TPC] chunks per tile
        self.groups = [list(range(i, min(cfg.TPC, i + cfg.G)))
                       for i in range(0, cfg.TPC, cfg.G)]
        self.cb = np.zeros(cfg.TPC, np.int64)
        self.gbase, self.gc = [], []
        c = 0
        for tl in self.groups:
            self.gbase.append(c)
            for t in tl:
                self.cb[t] = c
                c += nab[t]
            self.gc.append(c - self.gbase[-1])
        self.CT = c


def host_prep(cfg, edge_index, edge_attr):
    src = edge_index[0].astype(np.int64)
    dst = edge_index[1].astype(np.int64)
    cnt = np.bincount(dst, minlength=cfg.N).astype(np.float32)
    wp = (edge_attr.astype(np.float32)
          / np.maximum(cnt, 1.0)[dst]).astype(np.float32)

    loc = src % cfg.SHARD
    hsel = (loc >= cfg.SH2A).astype(np.int64)
    row = np.where(hsel == 0,
                   (src // cfg.SHARD) * cfg.SH2A + loc,
                   (src // cfg.SHARD) * cfg.SH2B + loc - cfg.SH2A)
    segkey = (dst >> 7) * 2 + hsel
    order = np.argsort(segkey, kind="stable")
    srow, sdst, swp = row[order], dst[order], wp[order]
    nseg = (cfg.NPAD // P) * 2
    seg_counts = np.bincount(segkey, minlength=nseg)
    seg_start = np.zeros(nseg + 1, np.int64)
    seg_start[1:] = np.cumsum(seg_counts)
    sc = seg_counts.reshape(cfg.NC, cfg.TPC, 2)
    nabAB = np.maximum(np.ceil(sc / P).astype(np.int64).max(axis=0), 1)
    plans = (PlanH(cfg, nabAB[:, 0]), PlanH(cfg, nabAB[:, 1]))

    idxs, ss = [], []
    for h in (0, 1):
        plan = plans[h]
        CT = plan.CT
        idx_arr = np.zeros((cfg.NC, 16, CT * 8), np.int16)
        s_arr = np.zeros((cfg.NC, P, CT, P), np.float32)
        for k in range(cfg.NC):
            for t in range(cfg.TPC):
                si = (k * cfg.TPC + t) * 2 + h
                i0, n = seg_start[si], seg_counts[si]
                if n == 0:
                    continue
                rows = srow[i0:i0 + n].astype(np.int16)
                stbase = k * cfg.SHARD + t * P
                dl = (sdst[i0:i0 + n] - stbase).astype(np.int64)
                cb = plan.cb[t]
                j = np.arange(n)
                s_arr[k, j % P, cb + j // P, dl] = swp[i0:i0 + n]
                idx_arr[k, j % 16, cb * 8 + j // 16] = rows
        idxs.append(np.tile(idx_arr, (1, 8, 1)))
        ss.append(s_arr)
    return plans, idxs, ss


def build_nc(cfg, plans):
    d0, d1, d2, d3 = cfg.D
    H1, H2 = d1 // P, d2 // P
    DT = BF16 if cfg.bf16 else F32
    planA, planB = plans
    NHA = cfg.NC * cfg.SH2A
    NHB = cfg.NC * cfg.SH2B

    nc = bacc.Bacc("TRN2", target_bir_lowering=False, debug=False,
                   num_devices=cfg.NC, enable_asserts=False,
                   num_swdge_queues=4)

    xA_in = nc.dram_tensor("xA", [NHA, d0], DT, kind="ExternalInput")
    xB_in = nc.dram_tensor("xB", [NHB, d0], DT, kind="ExternalInput")
    xT_in = nc.dram_tensor("xT", [P, cfg.SHARD], DT, kind="ExternalInput")
    idxA_in = nc.dram_tensor("idxA", [P, planA.CT * 8], I16,
                             kind="ExternalInput")
    idxB_in = nc.dram_tensor("idxB", [P, planB.CT * 8], I16,
                             kind="ExternalInput")
    sA_in = nc.dram_tensor("sA", [P, planA.CT, P], F8, kind="ExternalInput")
    sB_in = nc.dram_tensor("sB", [P, planB.CT, P], F8, kind="ExternalInput")
    wl0_in = nc.dram_tensor("wl0", [P, d1], DT, kind="ExternalInput")
    wr0_in = nc.dram_tensor("wr0", [P, d1], DT, kind="ExternalInput")
    wl1_in = nc.dram_tensor("wl1", [P, H1, d2], DT, kind="ExternalInput")
    wr1_in = nc.dram_tensor("wr1", [P, H1, d2], DT, kind="ExternalInput")
    wl2_in = nc.dram_tensor("wl2", [P, H2, d3], DT, kind="ExternalInput")
    wr2_in = nc.dram_tensor("wr2", [P, H2, d3], DT, kind="ExternalInput")
    b0_in = nc.dram_tensor("b0", [1, d1], DT, kind="ExternalInput")
    b1_in = nc.dram_tensor("b1", [1, d2], DT, kind="ExternalInput")
    b2_in = nc.dram_tensor("b2", [1, d3], DT, kind="ExternalInput")
    out_t = nc.dram_tensor("out", [cfg.SHARD, d3], F32, kind="ExternalOutput")

    with tile.TileContext(nc) as tc:
        with (
            tc.tile_pool(name="const", bufs=1) as cp,
            tc.tile_pool(name="mt", bufs=3) as mp,
            tc.tile_pool(name="st", bufs=3) as sp,
            tc.tile_pool(name="wk", bufs=3) as wk,
            tc.tile_pool(name="psum", bufs=2, space="PSUM") as ps,
            tc.tile_pool(name="dram", bufs=1, space="DRAM") as dr,
        ):
            h1ownA = dr.tile([cfg.SH2A, d1], F8)
            h1ownB = dr.tile([cfg.SH2B, d1], F8)
            h1fullA = dr.tile([NHA, d1], F8, addr_space="Shared")
            h1fullB = dr.tile([NHB, d1], F8, addr_space="Shared")
            h1T = dr.tile([d1, cfg.SHARD], DT)
            h2T = dr.tile([d2, cfg.SHARD], DT)
            h2pA = dr.tile([cfg.SH2A, cfg.EL3], DT)
            h2pB = dr.tile([cfg.SH2B, cfg.EL3], DT)
            h2pfullA = dr.tile([NHA, cfg.EL3], DT, addr_space="Shared")
            h2pfullB = dr.tile([NHB, cfg.EL3], DT, addr_space="Shared")

            # ---- constants / parameters
            ident_f = cp.tile([P, P], F32)
            make_identity(nc, ident_f[:])
            if cfg.bf16:
                ident_b = cp.tile([P, P], BF16)
                nc.vector.tensor_copy(ident_b[:], ident_f[:])
                ident_dt = ident_b
            else:
                ident_dt = ident_f
            ones_t = cp.tile([1, P], DT)
            nc.vector.memset(ones_t[:], 1.0)
            xT_t = cp.tile([P, cfg.SHARD], DT)
            nc.sync.dma_start(out=xT_t[:], in_=xT_in[:, :])
            idxA_t = cp.tile([P, planA.CT * 8], I16)
            nc.sync.dma_start(out=idxA_t[:], in_=idxA_in[:, :])
            idxB_t = cp.tile([P, planB.CT * 8], I16)
            nc.sync.dma_start(out=idxB_t[:], in_=idxB_in[:, :])
            wl0_t = cp.tile([P, d1], DT)
            nc.sync.dma_start(out=wl0_t[:], in_=wl0_in[:, :])
            wr0_t = cp.tile([P, d1], DT)
            nc.sync.dma_start(out=wr0_t[:], in_=wr0_in[:, :])
            wl1_t = cp.tile([P, H1, d2], DT)
            nc.sync.dma_start(out=wl1_t[:], in_=wl1_in[:, :, :])
            wr1_t = cp.tile([P, H1, d2], DT)
            nc.sync.dma_start(out=wr1_t[:], in_=wr1_in[:, :, :])
            wl2_t = cp.tile([P, H2, d3], DT)
            nc.sync.dma_start(out=wl2_t[:], in_=wl2_in[:, :, :])
            wr2_t = cp.tile([P, H2, d3], DT)
            nc.sync.dma_start(out=wr2_t[:], in_=wr2_in[:, :, :])
            b0_t = cp.tile([1, d1], DT)
            nc.sync.dma_start(out=b0_t[:], in_=b0_in[:, :])
            b1_t = cp.tile([1, d2], DT)
            nc.sync.dma_start(out=b1_t[:], in_=b1_in[:, :])
            b2_t = cp.tile([1, d3], DT)
            nc.sync.dma_start(out=b2_t[:], in_=b2_in[:, :])

            qctr = [0]  # round-robin SWDGE queue counter

            def gather_half(plan, idx_t, s_in, gi, table, elem, suf,
                            mdt=DT):
                gc = plan.gc[gi]
                gb = plan.gbase[gi]
                m_t = mp.tile([P, gc, elem], mdt, tag="mt" + suf)
                s_t = sp.tile([P, gc, P], F8, tag="st" + suf)
                hc = max(gc // 2, 1)
                nc.sync.dma_start(out=s_t[:, 0:hc, :],
                                  in_=s_in[:, gb:gb + hc, :])
                if gc > hc:
                    nc.sync.dma_start(out=s_t[:, hc:gc, :],
                                      in_=s_in[:, gb + hc:gb + gc, :])
                for t in plan.groups[gi]:
                    nch = int(plan.nab[t])
                    nidx = nch * P
                    cb = plan.cb[t]
                    nc.gpsimd.dma_gather(
                        m_t[:, cb - gb:cb - gb + nch, :], table,
                        idx_t[:, cb * 8:cb * 8 + nch * 8],
                        nidx, nidx, elem, single_packet=False,
                        queue_num=qctr[0] % 4)
                    qctr[0] += 1
                return m_t, s_t

            def ag(src_ap, dst_tile):
                nc.gpsimd.collective_compute(
                    "AllGather", Alu.bypass,
                    replica_groups=[list(range(cfg.NC))],
                    ins=[src_ap], outs=[dst_tile.opt()])

            TMID = cfg.TMID  # tiles 0..TMID-1 land in the A half

            # ---------------- Layer 1 ----------------
            ag1a_done = False
            for gi, tl in enumerate(planA.groups):
                mA, sAt = gather_half(planA, idxA_t, sA_in, gi,
                                      xA_in[:, :], d0, "A")
                mB, sBt = gather_half(planB, idxB_t, sB_in, gi,
                                      xB_in[:, :], d0, "B")
                gbA, gbB = planA.gbase[gi], planB.gbase[gi]
                for t in tl:
                    aggT = ps.tile([P, P], F32, tag="agg")
                    mm = ([(mA, sAt, c - gbA) for c in
                           range(planA.cb[t], planA.cb[t] + planA.nab[t])]
                          + [(mB, sBt, c - gbB) for c in
                             range(planB.cb[t], planB.cb[t] + planB.nab[t])])
                    for ci, (m_t, s_t, c) in enumerate(mm):
                        nc.tensor.matmul(
                            out=aggT[:], lhsT=m_t[:, c, :], rhs=s_t[:, c, :],
                            start=(ci == 0), stop=(ci == len(mm) - 1))
                    meanT = wk.tile([P, P], DT, tag="meanT")
                    nc.scalar.activation(meanT[:], aggT[:], Act.Copy)
                    op_ = ps.tile([P, d1], F32, tag="outp")
                    nc.tensor.matmul(out=op_[:], lhsT=meanT[:],
                                     rhs=wl0_t[:], start=True, stop=False)
                    nc.tensor.matmul(out=op_[:],
                                     lhsT=xT_t[:, t * P:(t + 1) * P],
                                     rhs=wr0_t[:], start=False, stop=False)
                    nc.tensor.matmul(out=op_[:], lhsT=ones_t[:],
                                     rhs=b0_t[:], start=False,
                                     stop=True, skip_group_check=True)
                    h_sb = wk.tile([P, d1], DT, tag="h_sb")
                    nc.vector.tensor_scalar(out=h_sb[:], in0=op_[:],
                                            scalar1=0.0, scalar2=None,
                                            op0=Alu.max)  # relu + cast
                    h8_sb = wk.tile([P, d1], F8, tag="h8")
                    nc.vector.tensor_scalar(out=h8_sb[:], in0=op_[:],
                                            scalar1=0.0, scalar2=None,
                                            op0=Alu.max)  # relu + fp8 cast
                    hodst, hor = ((h1ownA, t) if t < TMID
                                  else (h1ownB, t - TMID))
                    nc.sync.dma_start(
                        out=hodst[hor * P:(hor + 1) * P, :], in_=h8_sb[:])
                    for hh in range(H1):
                        trp = ps.tile([P, P], DT, tag="trp")
                        nc.tensor.transpose(
                            out=trp[:], in_=h_sb[:, hh * P:(hh + 1) * P],
                            identity=ident_dt[:])
                        hT_sb = wk.tile([P, P], DT, tag="hT_sb")
                        nc.vector.tensor_copy(hT_sb[:], trp[:])
                        nc.sync.dma_start(
                            out=h1T[hh * P:(hh + 1) * P, t * P:(t + 1) * P],
                            in_=hT_sb[:])
                if not ag1a_done and tl[-1] >= TMID - 1:
                    ag(h1ownA.opt(), h1fullA)
                    ag1a_done = True
            ag(h1ownB.opt(), h1fullB)

            # ---------------- Layer 2 ----------------
            accA2 = cp.tile([P, cfg.TPC, d1], DT)
            # pass A: first-half chunks only (overlaps AG1_B)
            for gi, tl in enumerate(planA.groups):
                mA, sAt = gather_half(planA, idxA_t, sA_in, gi,
                                      h1fullA[:, :], d1, "A", mdt=F8)
                gbA = planA.gbase[gi]
                for t in tl:
                    agg = ps.tile([P, d1], F32, tag="outp")
                    nab = int(planA.nab[t])
                    for ci in range(nab):
                        c = planA.cb[t] - gbA + ci
                        nc.tensor.matmul(
                            out=agg[:], lhsT=sAt[:, c, :], rhs=mA[:, c, :],
                            start=(ci == 0), stop=(ci == nab - 1))
                    nc.scalar.activation(accA2[:, t, :], agg[:], Act.Copy)
            # pass B: reload partial, add second-half chunks, finish layer
            ag2a_done = False
            for gi, tl in enumerate(planB.groups):
                mB, sBt = gather_half(planB, idxB_t, sB_in, gi,
                                      h1fullB[:, :], d1, "B", mdt=F8)
                gbB = planB.gbase[gi]
                for t in tl:
                    agg = ps.tile([P, d1], F32, tag="outp")
                    nc.tensor.matmul(out=agg[:], lhsT=ident_dt[:],
                                     rhs=accA2[:, t, :], start=True,
                                     stop=False)
                    nab = int(planB.nab[t])
                    for ci in range(nab):
                        c = planB.cb[t] - gbB + ci
                        nc.tensor.matmul(
                            out=agg[:], lhsT=sBt[:, c, :], rhs=mB[:, c, :],
                            start=False, stop=(ci == nab - 1))
                    agg_sb = wk.tile([P, d1], DT, tag="agg_sb")
                    nc.scalar.activation(agg_sb[:], agg[:], Act.Copy)
                    mts = []
                    for hh in range(H1):
                        trp = ps.tile([P, P], DT, tag="trp")
                        nc.tensor.transpose(
                            out=trp[:], in_=agg_sb[:, hh * P:(hh + 1) * P],
                            identity=ident_dt[:])
                        mt_sb = wk.tile([P, P], DT, tag="mT2")
                        nc.scalar.activation(mt_sb[:], trp[:], Act.Copy)
                        mts.append(mt_sb)
                    h1T_t = wk.tile([P, H1, P], DT, tag="hTt")
                    for hh in range(H1):
                        nc.sync.dma_start(
                            out=h1T_t[:, hh, :],
                            in_=h1T[hh * P:(hh + 1) * P, t * P:(t + 1) * P])
                    op_ = ps.tile([P, d2], F32, tag="outp")
                    for hh in range(H1):
                        nc.tensor.matmul(out=op_[:], lhsT=mts[hh][:],
                                         rhs=wl1_t[:, hh, :],
                                         start=(hh == 0), stop=False)
                    for hh in range(H1):
                        nc.tensor.matmul(out=op_[:], lhsT=h1T_t[:, hh, :],
                                         rhs=wr1_t[:, hh, :],
                                         start=False, stop=False)
                    nc.tensor.matmul(out=op_[:], lhsT=ones_t[:],
                                     rhs=b1_t[:], start=False, stop=True,
                                     skip_group_check=True)
                    h_sb = wk.tile([P, d2], DT, tag="h_sb")
                    nc.vector.tensor_scalar(out=h_sb[:], in0=op_[:],
                                            scalar1=0.0, scalar2=None,
                                            op0=Alu.max)  # relu + cast
                    hts = []
                    for hh in range(H2):
                        trp = ps.tile([P, P], DT, tag="trp")
                        nc.tensor.transpose(
                            out=trp[:], in_=h_sb[:, hh * P:(hh + 1) * P],
                            identity=ident_dt[:])
                        hT_sb = wk.tile([P, P], DT, tag="hT_sb")
                        nc.vector.tensor_copy(hT_sb[:], trp[:])
                        nc.sync.dma_start(
                            out=h2T[hh * P:(hh + 1) * P, t * P:(t + 1) * P],
                            in_=hT_sb[:])
                        hts.append(hT_sb)
                    prj = ps.tile([P, d3], F32, tag="proj")
                    for hh in range(H2):
                        nc.tensor.matmul(out=prj[:], lhsT=hts[hh][:],
                                         rhs=wl2_t[:, hh, :],
                                         start=(hh == 0), stop=(hh == H2 - 1))
                    prj_sb = wk.tile([P, d3], DT, tag="prj_sb")
                    nc.vector.tensor_copy(prj_sb[:], prj[:])
                    hpdst, hpr = ((h2pA, t) if t < TMID
                                  else (h2pB, t - TMID))
                    nc.sync.dma_start(
                        out=hpdst[hpr * P:(hpr + 1) * P, 0:d3],
                        in_=prj_sb[:])
                if not ag2a_done and tl[-1] >= TMID - 1:
                    ag(h2pA.opt(), h2pfullA)
                    ag2a_done = True
            ag(h2pB.opt(), h2pfullB)

            # ---------------- Layer 3 ----------------
            # log_softmax without max-subtraction (logit scale is small):
            # z - ln(sum(exp(z))); Exp accumulates per tile, batched Ln
            se_all = cp.tile([P, cfg.TPC], F32)
            z_all = cp.tile([P, cfg.TPC, d3], F32)
            ls_all = cp.tile([P, cfg.TPC], F32)
            accA3 = cp.tile([P, cfg.TPC, d3], DT)

            def emit_tail(t0, t1):
                nc.scalar.activation(ls_all[:, t0:t1], se_all[:, t0:t1],
                                     Act.Ln)
                for t in range(t0, t1):
                    out_sb = wk.tile([P, d3], F32, tag="out_sb")
                    nc.vector.tensor_scalar(out=out_sb[:],
                                            in0=z_all[:, t, :],
                                            scalar1=ls_all[:, t:t + 1],
                                            scalar2=None, op0=Alu.subtract)
                    nc.sync.dma_start(out=out_t[t * P:(t + 1) * P, :],
                                      in_=out_sb[:])

            # pass A (overlaps AG2_B)
            for gi, tl in enumerate(planA.groups):
                mA, sAt = gather_half(planA, idxA_t, sA_in, gi,
                                      h2pfullA[:, :], cfg.EL3, "A")
                gbA = planA.gbase[gi]
                for t in tl:
                    op_ = ps.tile([P, d3], F32, tag="proj")
                    nab = int(planA.nab[t])
                    for ci in range(nab):
                        c = planA.cb[t] - gbA + ci
                        nc.tensor.matmul(
                            out=op_[:], lhsT=sAt[:, c, :], rhs=mA[:, c, 0:d3],
                            start=(ci == 0), stop=(ci == nab - 1))
                    nc.vector.tensor_copy(accA3[:, t, :], op_[:])
            # pass B
            TS1 = (3 * cfg.TPC // 5) // cfg.G * cfg.G
            TS2 = (9 * cfg.TPC // 10) // cfg.G * cfg.G
            for gi, tl in enumerate(planB.groups):
                mB, sBt = gather_half(planB, idxB_t, sB_in, gi,
                                      h2pfullB[:, :], cfg.EL3, "B")
                gbB = planB.gbase[gi]
                for t in tl:
                    op_ = ps.tile([P, d3], F32, tag="proj")
                    nc.tensor.matmul(out=op_[:], lhsT=ident_dt[:],
                                     rhs=accA3[:, t, :], start=True,
                                     stop=False)
                    nab = int(planB.nab[t])
                    for ci in range(nab):
                        c = planB.cb[t] - gbB + ci
                        nc.tensor.matmul(
                            out=op_[:], lhsT=sBt[:, c, :], rhs=mB[:, c, 0:d3],
                            start=False, stop=False)
                    h2T_t = wk.tile([P, H2, P], DT, tag="hTt")
                    for hh in range(H2):
                        nc.sync.dma_start(
                            out=h2T_t[:, hh, :],
                            in_=h2T[hh * P:(hh + 1) * P, t * P:(t + 1) * P])
                    for hh in range(H2):
                        nc.tensor.matmul(out=op_[:], lhsT=h2T_t[:, hh, :],
                                         rhs=wr2_t[:, hh, :],
                                         start=False, stop=False,
                                         skip_group_check=True)
                    nc.tensor.matmul(out=op_[:], lhsT=ones_t[:],
                                     rhs=b2_t[:], start=False, stop=True,
                                     skip_group_check=True)
                    nc.vector.tensor_copy(z_all[:, t, :], op_[:])
                    e_dummy = wk.tile([P, d3], F32, tag="e_sb")
                    nc.scalar.activation(e_dummy[:], op_[:], Act.Exp,
                                         accum_out=se_all[:, t:t + 1])
                if tl[-1] + 1 == TS1:
                    emit_tail(0, TS1)
                elif tl[-1] + 1 == TS2:
                    emit_tail(TS1, TS2)
            emit_tail(TS2, cfg.TPC)

    nc.compile()
    return nc


_NC_CACHE = {}


def get_nc(cfg, plans):
    key = (cfg.key(), plans[0].nab.tobytes(), plans[1].nab.tobytes())
    if key not in _NC_CACHE:
        _NC_CACHE[key] = build_nc(cfg, plans)
    return _NC_CACHE[key]


def run(cfg, inputs, trace=False, tmpdir=None):
    x = np.asarray(inputs["x"], np.float32)
    plans, idxs, ss = host_prep(
        cfg, np.asarray(inputs["edge_index"]),
        np.asarray(inputs["edge_attr"], np.float32))
    d0, d1, d2, d3 = cfg.D
    H1, H2 = d1 // P, d2 // P
    npDT = np_bf16 if cfg.bf16 else np.float32

    xpad = np.zeros((cfg.NPAD, d0), np.float32)
    xpad[:cfg.N] = x
    xpad = xpad.astype(npDT)
    xsh = xpad.reshape(cfg.NC, cfg.SHARD, d0)
    xA = np.ascontiguousarray(xsh[:, :cfg.SH2A].reshape(-1, d0))
    xB = np.ascontiguousarray(xsh[:, cfg.SH2A:].reshape(-1, d0))
    Wl1 = np.asarray(inputs["Wl1"], np.float32)
    Wr1 = np.asarray(inputs["Wr1"], np.float32)
    Wl2 = np.asarray(inputs["Wl2"], np.float32)
    Wr2 = np.asarray(inputs["Wr2"], np.float32)
    shared = {
        "xA": xA,
        "xB": xB,
        "wl0": np.asarray(inputs["Wl0"], np.float32).astype(npDT),
        "wr0": np.asarray(inputs["Wr0"], np.float32).astype(npDT),
        "wl1": Wl1.reshape(H1, P, d2).transpose(1, 0, 2).astype(npDT),
        "wr1": Wr1.reshape(H1, P, d2).transpose(1, 0, 2).astype(npDT),
        "wl2": Wl2.reshape(H2, P, d3).transpose(1, 0, 2).astype(npDT),
        "wr2": Wr2.reshape(H2, P, d3).transpose(1, 0, 2).astype(npDT),
        "b0": (np.asarray(inputs["bl0"]) + np.asarray(inputs["br0"]))
        .astype(np.float32)[None, :].astype(npDT),
        "b1": (np.asarray(inputs["bl1"]) + np.asarray(inputs["br1"]))
        .astype(np.float32)[None, :].astype(npDT),
        "b2": (np.asarray(inputs["bl2"]) + np.asarray(inputs["br2"]))
        .astype(np.float32)[None, :].astype(npDT),
    }
    in_maps = []
    for k in range(cfg.NC):
        in_maps.append({
            **shared,
            "xT": np.ascontiguousarray(
                xpad[k * cfg.SHARD:(k + 1) * cfg.SHARD].T),
            "idxA": idxs[0][k],
            "idxB": idxs[1][k],
            "sA": ss[0][k].astype(np_f8),
            "sB": ss[1][k].astype(np_f8),
        })
    nc = get_nc(cfg, plans)
    res = run_bass_kernel_spmd(nc, in_maps, core_ids=list(range(cfg.NC)),
                               trace=trace, tmpdir=tmpdir)
    out = np.concatenate([res.results[k]["out"] for k in range(cfg.NC)],
                         axis=0)[:cfg.N]
    return np.ascontiguousarray(out.astype(np.float32)), res


def kernel(**inputs):
    cfg = Cfg()
    out, _ = run(cfg, inputs)
    return out


# revision 24
# speedup vs baseline: 1.1001x; 1.0151x over previous
"""Trainium2 Bass kernel: 3-layer edge-weighted GraphSAGE (Cluster-GCN style).

Strategy (8 NeuronCores, SPMD):
  - Nodes padded to NPAD = 8*SHARD, shard k = rows [k*SHARD, (k+1)*SHARD).
  - Edges sorted by (dst tile, src half); per dst-tile (128 nodes) the
    incoming edges' src rows are gathered with dma_gather (int16 indices,
    one call per (tile, half), rotated over the 4 SWDGE queues), then
    aggregated with a one-hot matmul into PSUM.
  - The one-hot selection matrices S[e, n] = (dst_e == n) * w'_e (with
    w' = edge_attr / max(indeg, 1), folding the mean) are identical for all
    three layers and are PRECOMPUTED ON THE HOST, uploaded to DRAM, and
    streamed into SBUF per chunk group (split into two DMAs per group so the
    large streaming descriptors don't head-of-line-block gather descriptors
    on the DMA engines).
  - A src node's "half" is whether it falls in the first or second half of
    its OWNING core's shard.  Each layer boundary then uses TWO AllGathers:
    AG_A for the first halves (issued mid-layer, as soon as the first
    TPC/2 tiles are done) and AG_B at the end of the layer.  The next
    layer runs in two passes: pass A aggregates only first-half chunks
    (tables ready after AG_A, so it overlaps AG_B), spilling the partial
    PSUM to SBUF; pass B reloads the partial via an identity matmul and
    finishes.  This hides nearly all collective time under gather work.
  - Layer 3 projects h2 @ Wl2 first (8 cols, padded to 256B rows) so its
    gather moves 256B/edge instead of 1KB/edge.
  - PSUM->SBUF copies are split between the Scalar engine (Act Copy only,
    so the activation table is loaded once) and the DVE; ReLU runs on the
    DVE (max with 0).  Layer 3's log_softmax skips the max-subtraction
    (|logits| is small) and batches: per-tile Exp with accumulate, batched
    Ln (avoids per-tile activation-table reloads at 1.28us each).
  - Full chunks are always gathered (padding slots -> row 0) so no SBUF
    garbage (possible NaN) reaches the PSUM accumulation through the zero
    columns of S.
  - bf16 matmul operands (fp32 PSUM accumulation) for full PE rate and
    half gather bandwidth.
"""
import numpy as np

import concourse.bacc as bacc
import concourse.tile as tile
from concourse import mybir
from concourse.bass_utils import run_bass_kernel_spmd
from concourse.masks import make_identity

from ml_dtypes import bfloat16 as np_bf16
from ml_dtypes import float8_e4m3fn as np_f8

F32 = mybir.dt.float32
BF16 = mybir.dt.bfloat16
F8 = mybir.dt.float8e4
I16 = mybir.dt.int16
P = 128
Alu = mybir.AluOpType
Act = mybir.ActivationFunctionType


class Cfg:
    def __init__(self, n_nodes=50000, n_edges=800000, dims=(128, 256, 256, 8),
                 ncores=8, G=2, bf16=True):
        self.N, self.E, self.D, self.NC = n_nodes, n_edges, dims, ncores
        self.SHARD = ((n_nodes + ncores * P - 1) // (ncores * P)) * P
        self.NPAD = self.SHARD * ncores
        self.TPC = self.SHARD // P
        self.TMID = (self.TPC + 1) // 2  # tiles in the A half
        self.SH2A = self.TMID * P
        self.SH2B = self.SHARD - self.SH2A
        assert self.NC * self.SH2A < 32768
        assert dims[0] == P and dims[1] % P == 0 and dims[2] % P == 0
        self.G, self.bf16 = G, bf16
        # L3 gather table row width (256B rows)
        self.EL3 = 128 if bf16 else 64

    def key(self):
        return (self.N, self.E, self.D, self.NC, self.G, self.bf16)


class PlanH:
    """Chunk layout for one src-half: per-tile chunk bases, group bases."""

    def __init__(self, cfg, nab):
        self.nab = nab  # [TPC] chunks per tile
        self.groups = [list(range(i, min(cfg.TPC, i + cfg.G)))
                       for i in range(0, cfg.TPC, cfg.G)]
        self.cb = np.zeros(cfg.TPC, np.int64)
        self.gbase, self.gc = [], []
        c = 0
        for tl in self.groups:
            self.gbase.append(c)
            for t in tl:
                self.cb[t] = c
                c += nab[t]
            self.gc.append(c - self.gbase[-1])
        self.CT = c


def host_prep(cfg, edge_index, edge_attr):
    src = edge_index[0].astype(np.int64)
    dst = edge_index[1].astype(np.int64)
    cnt = np.bincount(dst, minlength=cfg.N).astype(np.float32)
    wp = (edge_attr.astype(np.float32)
          / np.maximum(cnt, 1.0)[dst]).astype(np.float32)

    loc = src % cfg.SHARD
    hsel = (loc >= cfg.SH2A).astype(np.int64)
    row = np.where(hsel == 0,
                   (src // cfg.SHARD) * cfg.SH2A + loc,
                   (src // cfg.SHARD) * cfg.SH2B + loc - cfg.SH2A)
    segkey = (dst >> 7) * 2 + hsel
    order = np.argsort(segkey, kind="stable")
    srow, sdst, swp = row[order], dst[order], wp[order]
    nseg = (cfg.NPAD // P) * 2
    seg_counts = np.bincount(segkey, minlength=nseg)
    seg_start = np.zeros(nseg + 1, np.int64)
    seg_start[1:] = np.cumsum(seg_counts)
    sc = seg_counts.reshape(cfg.NC, cfg.TPC, 2)
    nabAB = np.maximum(np.ceil(sc / P).astype(np.int64).max(axis=0), 1)
    plans = (PlanH(cfg, nabAB[:, 0]), PlanH(cfg, nabAB[:, 1]))

    idxs, ss = [], []
    for h in (0, 1):
        plan = plans[h]
        CT = plan.CT
        idx_arr = np.zeros((cfg.NC, 16, CT * 8), np.int16)
        s_arr = np.zeros((cfg.NC, P, CT, P), np.float32)
        for k in range(cfg.NC):
            for t in range(cfg.TPC):
                si = (k * cfg.TPC + t) * 2 + h
                i0, n = seg_start[si], seg_counts[si]
                if n == 0:
                    continue
                rows = srow[i0:i0 + n].astype(np.int16)
                stbase = k * cfg.SHARD + t * P
                dl = (sdst[i0:i0 + n] - stbase).astype(np.int64)
                cb = plan.cb[t]
                j = np.arange(n)
                s_arr[k, j % P, cb + j // P, dl] = swp[i0:i0 + n]
                idx_arr[k, j % 16, cb * 8 + j // 16] = rows
        idxs.append(np.tile(idx_arr, (1, 8, 1)))
        ss.append(s_arr)
    return plans, idxs, ss


def build_nc(cfg, plans):
    d0, d1, d2, d3 = cfg.D
    H1, H2 = d1 // P, d2 // P
    DT = BF16 if cfg.bf16 else F32
    planA, planB = plans
    NHA = cfg.NC * cfg.SH2A
    NHB = cfg.NC * cfg.SH2B

    nc = bacc.Bacc("TRN2", target_bir_lowering=False, debug=False,
                   num_devices=cfg.NC, enable_asserts=False,
                   num_swdge_queues=4)

    xA_in = nc.dram_tensor("xA", [NHA, d0], DT, kind="ExternalInput")
    xB_in = nc.dram_tensor("xB", [NHB, d0], DT, kind="ExternalInput")
    xT_in = nc.dram_tensor("xT", [P, cfg.SHARD], DT, kind="ExternalInput")
    idxA_in = nc.dram_tensor("idxA", [P, planA.CT * 8], I16,
                             kind="ExternalInput")
    idxB_in = nc.dram_tensor("idxB", [P, planB.CT * 8], I16,
                             kind="ExternalInput")
    sA_in = nc.dram_tensor("sA", [P, planA.CT, P], F8, kind="ExternalInput")
    sB_in = nc.dram_tensor("sB", [P, planB.CT, P], F8, kind="ExternalInput")
    wl0_in = nc.dram_tensor("wl0", [P, d1], DT, kind="ExternalInput")
    wr0_in = nc.dram_tensor("wr0", [P, d1], DT, kind="ExternalInput")
    wl1_in = nc.dram_tensor("wl1", [P, H1, d2], DT, kind="ExternalInput")
    wr1_in = nc.dram_tensor("wr1", [P, H1, d2], DT, kind="ExternalInput")
    wl2_in = nc.dram_tensor("wl2", [P, H2, d3], DT, kind="ExternalInput")
    wr2_in = nc.dram_tensor("wr2", [P, H2, d3], DT, kind="ExternalInput")
    b0_in = nc.dram_tensor("b0", [1, d1], DT, kind="ExternalInput")
    b1_in = nc.dram_tensor("b1", [1, d2], DT, kind="ExternalInput")
    b2_in = nc.dram_tensor("b2", [1, d3], DT, kind="ExternalInput")
    out_t = nc.dram_tensor("out", [cfg.SHARD, d3], F32, kind="ExternalOutput")

    with tile.TileContext(nc) as tc:
        with (
            tc.tile_pool(name="const", bufs=1) as cp,
            tc.tile_pool(name="mt", bufs=4) as mp,
            tc.tile_pool(name="st", bufs=3) as sp,
            tc.tile_pool(name="wk", bufs=3) as wk,
            tc.tile_pool(name="psum", bufs=2, space="PSUM") as ps,
            tc.tile_pool(name="dram", bufs=1, space="DRAM") as dr,
        ):
            h1ownA = dr.tile([cfg.SH2A, d1], F8)
            h1ownB = dr.tile([cfg.SH2B, d1], F8)
            h1fullA = dr.tile([NHA, d1], F8, addr_space="Shared")
            h1fullB = dr.tile([NHB, d1], F8, addr_space="Shared")
            h1T = dr.tile([d1, cfg.SHARD], DT)
            h2T = dr.tile([d2, cfg.SHARD], DT)
            h2pA = dr.tile([cfg.SH2A, cfg.EL3], DT)
            h2pB = dr.tile([cfg.SH2B, cfg.EL3], DT)
            h2pfullA = dr.tile([NHA, cfg.EL3], DT, addr_space="Shared")
            h2pfullB = dr.tile([NHB, cfg.EL3], DT, addr_space="Shared")

            # ---- constants / parameters
            ident_f = cp.tile([P, P], F32)
            make_identity(nc, ident_f[:])
            if cfg.bf16:
                ident_b = cp.tile([P, P], BF16)
                nc.vector.tensor_copy(ident_b[:], ident_f[:])
                ident_dt = ident_b
            else:
                ident_dt = ident_f
            ones_t = cp.tile([1, P], DT)
            nc.vector.memset(ones_t[:], 1.0)
            xT_t = cp.tile([P, cfg.SHARD], DT)
            nc.sync.dma_start(out=xT_t[:], in_=xT_in[:, :])
            idxA_t = cp.tile([P, planA.CT * 8], I16)
            nc.sync.dma_start(out=idxA_t[:], in_=idxA_in[:, :])
            idxB_t = cp.tile([P, planB.CT * 8], I16)
            nc.sync.dma_start(out=idxB_t[:], in_=idxB_in[:, :])
            wl0_t = cp.tile([P, d1], DT)
            nc.sync.dma_start(out=wl0_t[:], in_=wl0_in[:, :])
            wr0_t = cp.tile([P, d1], DT)
            nc.sync.dma_start(out=wr0_t[:], in_=wr0_in[:, :])
            wl1_t = cp.tile([P, H1, d2], DT)
            nc.sync.dma_start(out=wl1_t[:], in_=wl1_in[:, :, :])
            wr1_t = cp.tile([P, H1, d2], DT)
            nc.sync.dma_start(out=wr1_t[:], in_=wr1_in[:, :, :])
            wl2_t = cp.tile([P, H2, d3], DT)
            nc.sync.dma_start(out=wl2_t[:], in_=wl2_in[:, :, :])
            wr2_t = cp.tile([P, H2, d3], DT)
            nc.sync.dma_start(out=wr2_t[:], in_=wr2_in[:, :, :])
            b0_t = cp.tile([1, d1], DT)
            nc.sync.dma_start(out=b0_t[:], in_=b0_in[:, :])
            b1_t = cp.tile([1, d2], DT)
            nc.sync.dma_start(out=b1_t[:], in_=b1_in[:, :])
            b2_t = cp.tile([1, d3], DT)
            nc.sync.dma_start(out=b2_t[:], in_=b2_in[:, :])

            # plan-A S matrices stay resident in SBUF for all 3 layers
            sAc = cp.tile([P, planA.CT, P], F8)
            for gi in range(len(planA.groups)):
                gb0, gc0 = planA.gbase[gi], planA.gc[gi]
                nc.sync.dma_start(out=sAc[:, gb0:gb0 + gc0, :],
                                  in_=sA_in[:, gb0:gb0 + gc0, :])

            qctr = [0]  # round-robin SWDGE queue counter

            def gather_half(plan, idx_t, s_in, gi, table, elem, suf,
                            mdt=DT):
                gc = plan.gc[gi]
                gb = plan.gbase[gi]
                m_t = mp.tile([P, gc, elem], mdt, tag="mt" + suf)
                if suf == "A":
                    s_t, s_off = sAc, 0  # resident, absolute chunk index
                else:
                    s_t = sp.tile([P, gc, P], F8, tag="st" + suf)
                    s_off = gb
                    hc = max(gc // 2, 1)
                    nc.sync.dma_start(out=s_t[:, 0:hc, :],
                                      in_=s_in[:, gb:gb + hc, :])
                    if gc > hc:
                        nc.sync.dma_start(out=s_t[:, hc:gc, :],
                                          in_=s_in[:, gb + hc:gb + gc, :])
                for t in plan.groups[gi]:
                    nch = int(plan.nab[t])
                    nidx = nch * P
                    cb = plan.cb[t]
                    nc.gpsimd.dma_gather(
                        m_t[:, cb - gb:cb - gb + nch, :], table,
                        idx_t[:, cb * 8:cb * 8 + nch * 8],
                        nidx, nidx, elem, single_packet=False,
                        queue_num=qctr[0] % 4)
                    qctr[0] += 1
                return m_t, s_t, s_off

            def ag(src_ap, dst_tile):
                nc.gpsimd.collective_compute(
                    "AllGather", Alu.bypass,
                    replica_groups=[list(range(cfg.NC))],
                    ins=[src_ap], outs=[dst_tile.opt()])

            TMID = cfg.TMID  # tiles 0..TMID-1 land in the A half

            # ---------------- Layer 1 ----------------
            ag1a_done = False
            for gi, tl in enumerate(planA.groups):
                mA, sAt, soA = gather_half(planA, idxA_t, sA_in, gi,
                                           xA_in[:, :], d0, "A")
                mB, sBt, soB = gather_half(planB, idxB_t, sB_in, gi,
                                           xB_in[:, :], d0, "B")
                gbA, gbB = planA.gbase[gi], planB.gbase[gi]
                for t in tl:
                    aggT = ps.tile([P, P], F32, tag="agg")
                    mm = ([(mA, sAt, c - gbA, c - soA) for c in
                           range(planA.cb[t], planA.cb[t] + planA.nab[t])]
                          + [(mB, sBt, c - gbB, c - soB) for c in
                             range(planB.cb[t], planB.cb[t] + planB.nab[t])])
                    for ci, (m_t, s_t, c, cs) in enumerate(mm):
                        nc.tensor.matmul(
                            out=aggT[:], lhsT=m_t[:, c, :], rhs=s_t[:, cs, :],
                            start=(ci == 0), stop=(ci == len(mm) - 1))
                    meanT = wk.tile([P, P], DT, tag="meanT")
                    nc.scalar.activation(meanT[:], aggT[:], Act.Copy)
                    op_ = ps.tile([P, d1], F32, tag="outp")
                    nc.tensor.matmul(out=op_[:], lhsT=meanT[:],
                                     rhs=wl0_t[:], start=True, stop=False)
                    nc.tensor.matmul(out=op_[:],
                                     lhsT=xT_t[:, t * P:(t + 1) * P],
                                     rhs=wr0_t[:], start=False, stop=False)
                    nc.tensor.matmul(out=op_[:], lhsT=ones_t[:],
                                     rhs=b0_t[:], start=False,
                                     stop=True, skip_group_check=True)
                    h_sb = wk.tile([P, d1], DT, tag="h_sb")
                    nc.vector.tensor_scalar(out=h_sb[:], in0=op_[:],
                                            scalar1=0.0, scalar2=None,
                                            op0=Alu.max)  # relu + cast
                    h8_sb = wk.tile([P, d1], F8, tag="h8")
                    nc.vector.tensor_scalar(out=h8_sb[:], in0=op_[:],
                                            scalar1=0.0, scalar2=None,
                                            op0=Alu.max)  # relu + fp8 cast
                    hodst, hor = ((h1ownA, t) if t < TMID
                                  else (h1ownB, t - TMID))
                    nc.sync.dma_start(
                        out=hodst[hor * P:(hor + 1) * P, :], in_=h8_sb[:])
                    for hh in range(H1):
                        trp = ps.tile([P, P], DT, tag="trp")
                        nc.tensor.transpose(
                            out=trp[:], in_=h_sb[:, hh * P:(hh + 1) * P],
                            identity=ident_dt[:])
                        hT_sb = wk.tile([P, P], DT, tag="hT_sb")
                        nc.vector.tensor_copy(hT_sb[:], trp[:])
                        nc.sync.dma_start(
                            out=h1T[hh * P:(hh + 1) * P, t * P:(t + 1) * P],
                            in_=hT_sb[:])
                if not ag1a_done and tl[-1] >= TMID - 1:
                    ag(h1ownA.opt(), h1fullA)
                    ag1a_done = True
            ag(h1ownB.opt(), h1fullB)

            # ---------------- Layer 2 ----------------
            accA2 = cp.tile([P, cfg.TPC, d1], DT)
            # pass A: first-half chunks only (overlaps AG1_B)
            for gi, tl in enumerate(planA.groups):
                mA, sAt, soA = gather_half(planA, idxA_t, sA_in, gi,
                                           h1fullA[:, :], d1, "A", mdt=F8)
                gbA = planA.gbase[gi]
                for t in tl:
                    agg = ps.tile([P, d1], F32, tag="outp")
                    nab = int(planA.nab[t])
                    for ci in range(nab):
                        c = planA.cb[t] - gbA + ci
                        cs = planA.cb[t] - soA + ci
                        nc.tensor.matmul(
                            out=agg[:], lhsT=sAt[:, cs, :], rhs=mA[:, c, :],
                            start=(ci == 0), stop=(ci == nab - 1))
                    nc.scalar.activation(accA2[:, t, :], agg[:], Act.Copy)
            # pass B: reload partial, add second-half chunks, finish layer
            ag2a_done = False
            for gi, tl in enumerate(planB.groups):
                mB, sBt, soB = gather_half(planB, idxB_t, sB_in, gi,
                                            h1fullB[:, :], d1, "B", mdt=F8)
                gbB = planB.gbase[gi]
                for t in tl:
                    agg = ps.tile([P, d1], F32, tag="outp")
                    nc.tensor.matmul(out=agg[:], lhsT=ident_dt[:],
                                     rhs=accA2[:, t, :], start=True,
                                     stop=False)
                    nab = int(planB.nab[t])
                    for ci in range(nab):
                        c = planB.cb[t] - gbB + ci
                        nc.tensor.matmul(
                            out=agg[:], lhsT=sBt[:, c, :], rhs=mB[:, c, :],
                            start=False, stop=(ci == nab - 1))
                    agg_sb = wk.tile([P, d1], DT, tag="agg_sb")
                    nc.scalar.activation(agg_sb[:], agg[:], Act.Copy)
                    mts = []
                    for hh in range(H1):
                        trp = ps.tile([P, P], DT, tag="trp")
                        nc.tensor.transpose(
                            out=trp[:], in_=agg_sb[:, hh * P:(hh + 1) * P],
                            identity=ident_dt[:])
                        mt_sb = wk.tile([P, P], DT, tag="mT2")
                        nc.scalar.activation(mt_sb[:], trp[:], Act.Copy)
                        mts.append(mt_sb)
                    h1T_t = wk.tile([P, H1, P], DT, tag="hTt")
                    for hh in range(H1):
                        nc.sync.dma_start(
                            out=h1T_t[:, hh, :],
                            in_=h1T[hh * P:(hh + 1) * P, t * P:(t + 1) * P])
                    op_ = ps.tile([P, d2], F32, tag="outp")
                    for hh in range(H1):
                        nc.tensor.matmul(out=op_[:], lhsT=mts[hh][:],
                                         rhs=wl1_t[:, hh, :],
                                         start=(hh == 0), stop=False)
                    for hh in range(H1):
                        nc.tensor.matmul(out=op_[:], lhsT=h1T_t[:, hh, :],
                                         rhs=wr1_t[:, hh, :],
                                         start=False, stop=False)
                    nc.tensor.matmul(out=op_[:], lhsT=ones_t[:],
                                     rhs=b1_t[:], start=False, stop=True,
                                     skip_group_check=True)
                    h_sb = wk.tile([P, d2], DT, tag="h_sb")
                    nc.vector.tensor_scalar(out=h_sb[:], in0=op_[:],
                                            scalar1=0.0, scalar2=None,
                                            op0=Alu.max)  # relu + cast
                    hts = []
                    for hh in range(H2):
                        trp = ps.tile([P, P], DT, tag="trp")
                        nc.tensor.transpose(
                            out=trp[:], in_=h_sb[:, hh * P:(hh + 1) * P],
                            identity=ident_dt[:])
                        hT_sb = wk.tile([P, P], DT, tag="hT_sb")
                        nc.vector.tensor_copy(hT_sb[:], trp[:])
                        nc.sync.dma_start(
                            out=h2T[hh * P:(hh + 1) * P, t * P:(t + 1) * P],
                            in_=hT_sb[:])
                        hts.append(hT_sb)
                    prj = ps.tile([P, d3], F32, tag="proj")
                    for hh in range(H2):
                        nc.tensor.matmul(out=prj[:], lhsT=hts[hh][:],
                                         rhs=wl2_t[:, hh, :],
                                         start=(hh == 0), stop=(hh == H2 - 1))
                    prj_sb = wk.tile([P, d3], DT, tag="prj_sb")
                    nc.vector.tensor_copy(prj_sb[:], prj[:])
                    hpdst, hpr = ((h2pA, t) if t < TMID
                                  else (h2pB, t - TMID))
                    nc.sync.dma_start(
                        out=hpdst[hpr * P:(hpr + 1) * P, 0:d3],
                        in_=prj_sb[:])
                if not ag2a_done and tl[-1] >= TMID - 1:
                    ag(h2pA.opt(), h2pfullA)
                    ag2a_done = True
            ag(h2pB.opt(), h2pfullB)

            # ---------------- Layer 3 ----------------
            # log_softmax without max-subtraction (logit scale is small):
            # z - ln(sum(exp(z))); Exp accumulates per tile, batched Ln
            se_all = cp.tile([P, cfg.TPC], F32)
            z_all = cp.tile([P, cfg.TPC, d3], F32)
            ls_all = cp.tile([P, cfg.TPC], F32)
            accA3 = cp.tile([P, cfg.TPC, d3], DT)

            def emit_tail(t0, t1):
                nc.scalar.activation(ls_all[:, t0:t1], se_all[:, t0:t1],
                                     Act.Ln)
                for t in range(t0, t1):
                    out_sb = wk.tile([P, d3], F32, tag="out_sb")
                    nc.vector.tensor_scalar(out=out_sb[:],
                                            in0=z_all[:, t, :],
                                            scalar1=ls_all[:, t:t + 1],
                                            scalar2=None, op0=Alu.subtract)
                    nc.sync.dma_start(out=out_t[t * P:(t + 1) * P, :],
                                      in_=out_sb[:])

            # pass A (overlaps AG2_B)
            for gi, tl in enumerate(planA.groups):
                mA, sAt, soA = gather_half(planA, idxA_t, sA_in, gi,
                                           h2pfullA[:, :], cfg.EL3, "A")
                gbA = planA.gbase[gi]
                for t in tl:
                    op_ = ps.tile([P, d3], F32, tag="proj")
                    nab = int(planA.nab[t])
                    for ci in range(nab):
                        c = planA.cb[t] - gbA + ci
                        cs = planA.cb[t] - soA + ci
                        nc.tensor.matmul(
                            out=op_[:], lhsT=sAt[:, cs, :], rhs=mA[:, c, 0:d3],
                            start=(ci == 0), stop=(ci == nab - 1))
                    nc.vector.tensor_copy(accA3[:, t, :], op_[:])
            # pass B
            TS1 = (3 * cfg.TPC // 5) // cfg.G * cfg.G
            TS2 = (9 * cfg.TPC // 10) // cfg.G * cfg.G
            for gi, tl in enumerate(planB.groups):
                mB, sBt, soB = gather_half(planB, idxB_t, sB_in, gi,
                                            h2pfullB[:, :], cfg.EL3, "B")
                gbB = planB.gbase[gi]
                for t in tl:
                    op_ = ps.tile([P, d3], F32, tag="proj")
                    nc.tensor.matmul(out=op_[:], lhsT=ident_dt[:],
                                     rhs=accA3[:, t, :], start=True,
                                     stop=False)
                    nab = int(planB.nab[t])
                    for ci in range(nab):
                        c = planB.cb[t] - gbB + ci
                        nc.tensor.matmul(
                            out=op_[:], lhsT=sBt[:, c, :], rhs=mB[:, c, 0:d3],
                            start=False, stop=False)
                    h2T_t = wk.tile([P, H2, P], DT, tag="hTt")
                    for hh in range(H2):
                        nc.sync.dma_start(
                            out=h2T_t[:, hh, :],
                            in_=h2T[hh * P:(hh + 1) * P, t * P:(t + 1) * P])
                    for hh in range(H2):
                        nc.tensor.matmul(out=op_[:], lhsT=h2T_t[:, hh, :],
                                         rhs=wr2_t[:, hh, :],
                                         start=False, stop=False,
                                         skip_group_check=True)
                    nc.tensor.matmul(out=op_[:], lhsT=ones_t[:],
                                     rhs=b2_t[:], start=False, stop=True,
                                     skip_group_check=True)
                    nc.vector.tensor_copy(z_all[:, t, :], op_[:])
                    e_dummy = wk.tile([P, d3], F32, tag="e_sb")
                    nc.scalar.activation(e_dummy[:], op_[:], Act.Exp,
                                         accum_out=se_all[:, t:t + 1])
                if tl[-1] + 1 == TS1:
                    emit_tail(0, TS1)
                elif tl[-1] + 1 == TS2:
                    emit_tail(TS1, TS2)
            emit_tail(TS2, cfg.TPC)

    nc.compile()
    return nc


_NC_CACHE = {}


def get_nc(cfg, plans):
    key = (cfg.key(), plans[0].nab.tobytes(), plans[1].nab.tobytes())
    if key not in _NC_CACHE:
        _NC_CACHE[key] = build_nc(cfg, plans)
    return _NC_CACHE[key]


def run(cfg, inputs, trace=False, tmpdir=None):
    x = np.asarray(inputs["x"], np.float32)
    plans, idxs, ss = host_prep(
        cfg, np.asarray(inputs["edge_index"]),
        np.asarray(inputs["edge_attr"], np.float32))
    d0, d1, d2, d3 = cfg.D
    H1, H2 = d1 // P, d2 // P
    npDT = np_bf16 if cfg.bf16 else np.float32

    xpad = np.zeros((cfg.NPAD, d0), np.float32)
    xpad[:cfg.N] = x
    xpad = xpad.astype(npDT)
    xsh = xpad.reshape(cfg.NC, cfg.SHARD, d0)
    xA = np.ascontiguousarray(xsh[:, :cfg.SH2A].reshape(-1, d0))
    xB = np.ascontiguousarray(xsh[:, cfg.SH2A:].reshape(-1, d0))
    Wl1 = np.asarray(inputs["Wl1"], np.float32)
    Wr1 = np.asarray(inputs["Wr1"], np.float32)
    Wl2 = np.asarray(inputs["Wl2"], np.float32)
    Wr2 = np.asarray(inputs["Wr2"], np.float32)
    shared = {
        "xA": xA,
        "xB": xB,
        "wl0": np.asarray(inputs["Wl0"], np.float32).astype(npDT),
        "wr0": np.asarray(inputs["Wr0"], np.float32).astype(npDT),
        "wl1": Wl1.reshape(H1, P, d2).transpose(1, 0, 2).astype(npDT),
        "wr1": Wr1.reshape(H1, P, d2).transpose(1, 0, 2).astype(npDT),
        "wl2": Wl2.reshape(H2, P, d3).transpose(1, 0, 2).astype(npDT),
        "wr2": Wr2.reshape(H2, P, d3).transpose(1, 0, 2).astype(npDT),
        "b0": (np.asarray(inputs["bl0"]) + np.asarray(inputs["br0"]))
        .astype(np.float32)[None, :].astype(npDT),
        "b1": (np.asarray(inputs["bl1"]) + np.asarray(inputs["br1"]))
        .astype(np.float32)[None, :].astype(npDT),
        "b2": (np.asarray(inputs["bl2"]) + np.asarray(inputs["br2"]))
        .astype(np.float32)[None, :].astype(npDT),
    }
    in_maps = []
    for k in range(cfg.NC):
        in_maps.append({
            **shared,
            "xT": np.ascontiguousarray(
                xpad[k * cfg.SHARD:(k + 1) * cfg.SHARD].T),
            "idxA": idxs[0][k],
            "idxB": idxs[1][k],
            "sA": ss[0][k].astype(np_f8),
            "sB": ss[1][k].astype(np_f8),
        })
    nc = get_nc(cfg, plans)
    res = run_bass_kernel_spmd(nc, in_maps, core_ids=list(range(cfg.NC)),
                               trace=trace, tmpdir=tmpdir)
    out = np.concatenate([res.results[k]["out"] for k in range(cfg.NC)],
                         axis=0)[:cfg.N]
    return np.ascontiguousarray(out.astype(np.float32)), res


def kernel(**inputs):
    cfg = Cfg()
    out, _ = run(cfg, inputs)
    return out
